# revision 1
# baseline (speedup 1.0000x reference)
"""Multi-head graph attention on 8 Trainium2 NeuronCores.

Strategy: shard destination nodes (and their incoming edges) across the 8
cores; every core redundantly projects the full node table (k|v, bf16) into
its HBM plus q for its own dst range, then streams its dst-sorted edges:
  - k|v and q rows fetched with gpsimd.dma_gather (4 SWDGE queues)
  - edge_attr projected on PE (bias fused as an extra contraction row)
  - logits/exp on DVE+ACT; numerator|denominator scatter-added per
    128-dst-node block via one-hot matmul accumulation in PSUM
  - normalize, PE-transpose, project through Wo, emit transposed output
No collectives: each core owns its output rows exclusively.
"""

import numpy as np
import ml_dtypes

D, H, ED = 128, 8, 64
DH = D // H
SCALE = DH ** -0.5
F32 = np.float32
BF16 = ml_dtypes.bfloat16


class Cfg:
    def __init__(self, N=50000, E=600000, ncores=8, split=32768, chunk=8):
        self.N, self.E, self.NCORES = N, E, ncores
        self.NPC = N // ncores
        self.NBLK = (self.NPC + 127) // 128
        self.NPAD = self.NBLK * 128
        self.SPLIT = split
        assert split % 512 == 0
        self.LO_G = split // 512
        self.HI_G = (N - split + 511) // 512
        self.HI_ROWS = self.HI_G * 512
        self.XT_COLS = (self.LO_G + self.HI_G) * 512
        self.QG = (self.NPAD + 511) // 512
        self.Q_ROWS = self.QG * 512
        self.CHUNK = chunk


CFG = Cfg()


def _wrap_idx(vals):
    """idx list (len = multiple of 128) -> [128, len//16] int16 in the SWDGE
    wrapped layout: index i at (partition i%16, col i//16), replicated to all
    eight 16-partition groups."""
    n = len(vals)
    blk = np.asarray(vals, np.int16).reshape(n // 16, 16).T
    return np.tile(blk, (8, 1))


def _preprocess(edge_index, cfg=CFG):
    src = np.asarray(edge_index[0], np.int64)
    dst = np.asarray(edge_index[1], np.int64)
    order = np.argsort(dst, kind="stable")
    src_s, dst_s, eid_s = src[order], dst[order], order

    core_of = dst_s // cfg.NPC
    per = [[None] * cfg.NBLK for _ in range(cfg.NCORES)]
    for c in range(cfg.NCORES):
        m = core_of == c
        sc, dc, ec = src_s[m], dst_s[m], eid_s[m]
        ld = dc - c * cfg.NPC
        blk = ld // 128
        for b in range(cfg.NBLK):
            mb = blk == b
            sb, lb, eb = sc[mb], ld[mb] - b * 128, ec[mb]
            lo = sb < cfg.SPLIT
            per[c][b] = (sb[lo], lb[lo], eb[lo], sb[~lo], lb[~lo], eb[~lo])

    Lb = [max(1, max((len(per[c][b][0]) + 127) // 128 for c in range(cfg.NCORES)))
          for b in range(cfg.NBLK)]
    Hb = [max((len(per[c][b][3]) + 127) // 128 for c in range(cfg.NCORES))
          for b in range(cfg.NBLK)]
    Tb = [Lb[b] + Hb[b] for b in range(cfg.NBLK)]
    T = sum(Tb)

    kv_calls = [[] for _ in range(cfg.NBLK)]
    q_calls = [[] for _ in range(cfg.NBLK)]
    kvw = qw = 0
    for b in range(cfg.NBLK):
        for t0 in range(0, Lb[b], cfg.CHUNK):
            nt = min(cfg.CHUNK, Lb[b] - t0)
            kv_calls[b].append(("lo", t0, nt, kvw)); kvw += nt * 8
        for t0 in range(0, Hb[b], cfg.CHUNK):
            nt = min(cfg.CHUNK, Hb[b] - t0)
            kv_calls[b].append(("hi", Lb[b] + t0, nt, kvw)); kvw += nt * 8
        for t0 in range(0, Tb[b], cfg.CHUNK):
            nt = min(cfg.CHUNK, Tb[b] - t0)
            q_calls[b].append((t0, nt, qw)); qw += nt * 8

    kvidx = np.zeros((cfg.NCORES, 128, kvw), np.int16)
    qidx = np.zeros((cfg.NCORES, 128, qw), np.int16)
    ld_all = np.full((cfg.NCORES, 128, T), -1.0, F32)
    eids = np.full((cfg.NCORES, T, 128), -1, np.int64)
    for c in range(cfg.NCORES):
        gt = 0
        for b in range(cfg.NBLK):
            slo, llo, elo, shi, lhi, ehi = per[c][b]
            nlo, nhi = Lb[b] * 128, Hb[b] * 128
            kvv = np.zeros(nlo + nhi, np.int64)
            qv = np.zeros(nlo + nhi, np.int64)
            lv = np.full(nlo + nhi, -1.0, F32)
            ev = np.full(nlo + nhi, -1, np.int64)
            kvv[:len(slo)] = slo
            kvv[nlo:nlo + len(shi)] = shi - cfg.SPLIT
            qv[:len(llo)] = llo + b * 128
            qv[nlo:nlo + len(lhi)] = lhi + b * 128
            lv[:len(llo)] = llo
            lv[nlo:nlo + len(lhi)] = lhi
            ev[:len(elo)] = elo
            ev[nlo:nlo + len(ehi)] = ehi
            for (tab, t0, nt, off) in kv_calls[b]:
                kvidx[c][:, off:off + nt * 8] = _wrap_idx(
                    kvv[t0 * 128: (t0 + nt) * 128])
            for (t0, nt, off) in q_calls[b]:
                qidx[c][:, off:off + nt * 8] = _wrap_idx(
                    qv[t0 * 128: (t0 + nt) * 128])
            ld_all[c][:, gt:gt + Tb[b]] = lv.reshape(Tb[b], 128).T
            eids[c][gt:gt + Tb[b]] = ev.reshape(Tb[b], 128)
            gt += Tb[b]

    return dict(Lb=Lb, Hb=Hb, Tb=Tb, T=T, kv_calls=kv_calls, q_calls=q_calls,
                kvw=kvw, qw=qw, kvidx=kvidx, qidx=qidx, ld_all=ld_all,
                eids=eids)


def _build_program(plan, cfg=CFG, repeat=1, parts="p0,ea,gath,tiles,norm,proj",
                   rep_barrier=False):
    import concourse.bacc as bacc
    import concourse.tile as tile
    import concourse.bass as bass
    import concourse.mybir as mybir

    f32, bf16, i16 = mybir.dt.float32, mybir.dt.bfloat16, mybir.dt.int16
    Alu, Act = mybir.AluOpType, mybir.ActivationFunctionType
    T, Tb = plan["T"], plan["Tb"]
    P = set(parts.split(","))

    nc = bacc.Bacc("TRN2", target_bir_lowering=False, debug=False,
                   enable_asserts=False, num_devices=cfg.NCORES,
                   num_swdge_queues=4)

    def din(name, shape, dt):
        return nc.dram_tensor(name, list(shape), dt, kind="ExternalInput").ap()

    xT = din("xT", [128, cfg.XT_COLS], bf16)
    xTq = din("xTq", [128, cfg.Q_ROWS], bf16)
    Wkv = din("Wkv", [128, 256], bf16)
    Wq_ = din("Wq_", [128, 128], bf16)
    We65 = din("We65", [65, 128], bf16)
    Wo_ = din("Wo_", [128, 128], f32)
    bkv = din("bkv", [1, 256], bf16)
    bq_ = din("bq_", [1, 128], bf16)
    bo_row = din("bo_row", [1, 128], f32)
    ones_row = din("ones_row", [1, 512], f32)
    ones16_row = din("ones16_row", [1, 128], bf16)
    iota_in = din("iota_in", [128, 128], bf16)
    ident_in = din("ident_in", [128, 128], f32)
    ident16_in = din("ident16_in", [128, 128], bf16)
    ea_all = din("ea_all", [T, 65, 128], bf16)
    ld_in = din("ld_in", [128, T], bf16)
    kvidx_in = din("kvidx_in", [128, plan["kvw"]], i16)
    qidx_in = din("qidx_in", [128, plan["qw"]], i16)
    outT = nc.dram_tensor("outT", [128, cfg.NPAD], f32,
                          kind="ExternalOutput").ap()

    kv_lo = nc.dram_tensor("kv_lo", [cfg.SPLIT, 256], bf16, kind="Internal").ap()
    kv_hi = nc.dram_tensor("kv_hi", [cfg.HI_ROWS, 256], bf16,
                           kind="Internal").ap()
    q_tab = nc.dram_tensor("q_tab", [cfg.Q_ROWS, 128], bf16,
                           kind="Internal").ap()

    def vw(a, dims, off=0):
        """View AP `a` with replaced free dims [[step, count], ...] and an
        extra element offset into the free space."""
        return bass.AP(a.tensor, a.offset + off,
                       [list(a.ap[0])] + [list(d) for d in dims])

    def dap(a, dims, off=0):
        """Raw AP on tensor of `a` with fully explicit dims."""
        return bass.AP(a.tensor, a.offset + off, [list(d) for d in dims])

    with tile.TileContext(nc) as tc:
        with tc.tile_pool(name="const", bufs=1) as cpool:
            def cin(tag, shape, dt, src):
                t = cpool.tile(shape, dt, tag=tag)
                nc.sync.dma_start(out=t[:], in_=src)
                return t

            Wkv_sb = cin("Wkv", [128, 256], bf16, Wkv[:])
            Wq_sb = cin("Wq", [128, 128], bf16, Wq_[:])
            We_sb = cin("We", [65, 128], bf16, We65[:])
            Wo_sb = cin("Wo", [128, 128], f32, Wo_[:])
            bkv_sb = cin("bkv", [1, 256], bf16, bkv[:])
            bq_sb = cin("bq", [1, 128], bf16, bq_[:])
            bo_sb = cin("bo", [1, 128], f32, bo_row[:])
            ones_sb = cin("ones", [1, 512], f32, ones_row[:])
            ones16_sb = cin("ones16", [1, 128], bf16, ones16_row[:])
            iota_sb = cin("iota", [128, 128], bf16, iota_in[:])
            ident_sb = cin("ident", [128, 128], f32, ident_in[:])
            ident16_sb = cin("ident16", [128, 128], bf16, ident16_in[:])
            ld_sb = cin("ld", [128, T], bf16, ld_in[:])
            kvidx_sb = cin("kvidx", [128, plan["kvw"]], i16, kvidx_in[:])
            qidx_sb = cin("qidx", [128, plan["qw"]], i16, qidx_in[:])
            oT_all = cpool.tile([128, cfg.NPAD], f32, tag="oT_all")

            for _rep in range(repeat):
                # ---------------- phase 0: node projections ----------------
                with tc.tile_pool(name="p0", bufs=3) as p0, \
                     tc.tile_pool(name="p0ps", bufs=4, space="PSUM") as p0ps:
                    def proj(g, src_ap, w_sb, b_sb, width, table):
                        xt = p0.tile([128, 512], bf16, tag="xt")
                        nc.sync.dma_start(out=xt[:], in_=src_ap)
                        o4 = p0.tile([128, 4, width], bf16, tag="o4")
                        for i in range(4):
                            ps = p0ps.tile([128, width], f32, tag="ps")
                            nc.tensor.matmul(out=ps[:],
                                             lhsT=xt[:, i * 128:(i + 1) * 128],
                                             rhs=w_sb[:], start=True, stop=False)
                            nc.tensor.matmul(out=ps[:], lhsT=ones16_sb[:],
                                             rhs=b_sb[:], start=False, stop=True)
                            if i % 2 == 0:
                                nc.vector.tensor_copy(out=o4[:, i, :], in_=ps[:])
                            else:
                                nc.scalar.activation(out=o4[:, i, :], in_=ps[:],
                                                     func=Act.Copy)
                        dst = dap(table, [[width, 128], [128 * width, 4],
                                          [1, width]], off=g * 512 * width)
                        nc.sync.dma_start(out=dst, in_=o4[:])

                    for g in range(cfg.LO_G if "p0" in P else 0):
                        proj(g, xT[:, g * 512:(g + 1) * 512], Wkv_sb, bkv_sb, 256,
                             kv_lo)
                    for g in range(cfg.HI_G if "p0" in P else 0):
                        proj(g, xT[:, (cfg.LO_G + g) * 512:(cfg.LO_G + g + 1) * 512],
                             Wkv_sb, bkv_sb, 256, kv_hi)
                    for g in range(cfg.QG if "p0" in P else 0):
                        proj(g, xTq[:, g * 512:(g + 1) * 512], Wq_sb, bq_sb, 128,
                             q_tab)

                # ---------------- phase 1: edge pass ----------------
                with tc.tile_pool(name="blk", bufs=2) as blkp, \
                     tc.tile_pool(name="wk", bufs=3) as wk, \
                     tc.tile_pool(name="nrm", bufs=2) as nrm, \
                     tc.tile_pool(name="ps1", bufs=2, space="PSUM") as ps1, \
                     tc.tile_pool(name="ps2", bufs=3, space="PSUM") as ps2:
                    qrr = [0]

                    def nextq():
                        qrr[0] = (qrr[0] + 1) % 4
                        return qrr[0]

                    gt = 0
                    for b in range(cfg.NBLK):
                        tb = Tb[b]
                        kv_g = blkp.tile([128, tb, 256], bf16, tag="kv_g")
                        q_g = blkp.tile([128, tb, 128], bf16, tag="q_g")
                        ea_sb = blkp.tile([65, tb * 128], bf16, tag="ea")
                        if "ea" in P:
                            nc.scalar.dma_start(
                                out=vw(ea_sb[:], [[128, tb], [1, 128]]),
                                in_=dap(ea_all,
                                        [[128, 65], [65 * 128, tb], [1, 128]],
                                        off=gt * 65 * 128))
                        for (tab, t0, nt, off) in (plan["kv_calls"][b] if "gath" in P else []):
                            table = kv_lo if tab == "lo" else kv_hi
                            nc.gpsimd.dma_gather(
                                out_ap=kv_g[:, t0:t0 + nt, :], in_ap=table[:],
                                idxs_ap=kvidx_sb[:, off:off + nt * 8],
                                num_idxs=nt * 128, num_idxs_reg=nt * 128,
                                elem_size=256, queue_num=nextq())
                        for (t0, nt, off) in (plan["q_calls"][b] if "gath" in P else []):
                            nc.gpsimd.dma_gather(
                                out_ap=q_g[:, t0:t0 + nt, :], in_ap=q_tab[:],
                                idxs_ap=qidx_sb[:, off:off + nt * 8],
                                num_idxs=nt * 128, num_idxs_reg=nt * 128,
                                elem_size=128, queue_num=nextq())

                        ud = ps1.tile([128, 136], f32, tag="ud")
                        ntile = 0
                        for g0 in range(0, tb if "tiles" in P else 0, 4):
                            nt = min(4, tb - g0)
                            eps = ps2.tile([128, 512], f32, tag="eps")
                            for i in range(nt):
                                nc.tensor.matmul(
                                    out=eps[:, i * 128:(i + 1) * 128],
                                    lhsT=ea_sb[:, (g0 + i) * 128:(g0 + i + 1) * 128],
                                    rhs=We_sb[:], start=True, stop=False)
                                nc.tensor.matmul(
                                    out=eps[:, i * 128:(i + 1) * 128],
                                    lhsT=ident16_sb[:],
                                    rhs=kv_g[:, g0 + i, 0:128],
                                    start=False, stop=True,
                                    skip_group_check=True)
                            qw4 = wk.tile([128, 4, 128], f32, tag="qw4")
                            nc.vector.tensor_tensor(
                                out=vw(qw4[:], [[1, nt * 128]]),
                                in0=vw(q_g[:], [[1, nt * 128]], off=g0 * 128),
                                in1=vw(eps[:], [[1, nt * 128]]),
                                op=Alu.mult)
                            l4 = wk.tile([128, 32], f32, tag="l4")
                            nc.vector.reduce_sum(
                                out=vw(l4[:], [[1, nt * 8]]),
                                in_=vw(qw4[:], [[16, nt * 8], [1, 16]]),
                                axis=mybir.AxisListType.X)
                            rhs4 = wk.tile([128, 4, 136], bf16, tag="rhs4")
                            nc.scalar.activation(
                                out=vw(rhs4[:], [[136, nt], [1, 8]], off=128),
                                in_=vw(l4[:], [[8, nt], [1, 8]]),
                                func=Act.Exp, scale=float(SCALE))
                            nc.vector.tensor_tensor(
                                out=vw(rhs4[:], [[136, nt], [16, 8], [1, 16]]),
                                in0=vw(kv_g[:], [[256, nt], [16, 8], [1, 16]],
                                       off=g0 * 256 + 128),
                                in1=vw(rhs4[:], [[136, nt], [1, 8], [0, 16]],
                                       off=128),
                                op=Alu.mult)
                            S4 = wk.tile([128, 4, 128], bf16, tag="S4")
                            nc.vector.tensor_tensor(
                                out=S4[:, :nt, :],
                                in0=vw(iota_sb[:], [[0, nt], [1, 128]]),
                                in1=vw(ld_sb[:], [[1, nt], [0, 128]], off=gt + g0),
                                op=Alu.is_equal)
                            for i in range(nt):
                                nc.tensor.matmul(out=ud[:], lhsT=S4[:, i, :],
                                                 rhs=rhs4[:, i, :],
                                                 start=(ntile == 0),
                                                 stop=(ntile == tb - 1),
                                                 skip_group_check=True)
                                ntile += 1
                        if "norm" not in P:
                            gt += tb
                            continue
                        d8 = nrm.tile([128, 8], f32, tag="d8")
                        nc.scalar.activation(out=d8[:], in_=ud[:, 128:136],
                                             func=Act.Copy)
                        nc.vector.tensor_scalar_max(d8[:], d8[:], 1e-30)
                        r8 = nrm.tile([128, 8], f32, tag="r8")
                        nc.vector.reciprocal(r8[:], d8[:])
                        o_sb = nrm.tile([128, 128], f32, tag="o_sb")
                        nc.vector.tensor_tensor(
                            out=vw(o_sb[:], [[16, 8], [1, 16]]),
                            in0=vw(ud[:], [[16, 8], [1, 16]]),
                            in1=vw(r8[:], [[1, 8], [0, 16]]),
                            op=Alu.mult)
                        oT_ps = ps2.tile([128, 128], f32, tag="oT")
                        nc.tensor.transpose(out=oT_ps[:], in_=o_sb[:],
                                            identity=ident_sb[:])
                        nc.scalar.activation(out=oT_all[:, b * 128:(b + 1) * 128],
                                             in_=oT_ps[:], func=Act.Copy)
                        gt += tb

                    # ---------------- output projection ----------------
                    for j in range((cfg.NPAD + 511) // 512 if "proj" in P else 0):
                        w = min(512, cfg.NPAD - j * 512)
                        pps = ps2.tile([128, 512], f32, tag="oT")
                        nc.tensor.matmul(out=pps[:, :w], lhsT=Wo_sb[:],
                                         rhs=oT_all[:, j * 512:j * 512 + w],
                                         start=True, stop=False)
                        nc.tensor.matmul(out=pps[:, :w], lhsT=bo_sb[:],
                                         rhs=ones_sb[:, :w], start=False, stop=True)
                        ot = wk.tile([128, 512], f32, tag="ot")
                        nc.vector.tensor_copy(out=ot[:, :w], in_=pps[:, :w])
                        nc.sync.dma_start(out=outT[:, j * 512:j * 512 + w],
                                          in_=ot[:, :w])
                    if rep_barrier:
                        tc.strict_bb_all_engine_barrier()

    nc.compile()
    return nc


def _make_inputs(plan, x, edge_attr, Wq, bq, Wk, bk, Wv, bv, We, be, Wo, bo,
                 cfg=CFG):
    x = np.asarray(x, F32)
    ea = np.asarray(edge_attr, F32)
    xT_pad = np.zeros((128, cfg.XT_COLS), BF16)
    xT_pad[:, :cfg.N] = np.ascontiguousarray(x.T).astype(BF16)
    Wkv = np.concatenate([np.asarray(Wk, F32), np.asarray(Wv, F32)],
                         axis=1).astype(BF16)
    We65 = np.concatenate([np.asarray(We, F32), np.asarray(be, F32)[None, :]],
                          axis=0).astype(BF16)
    bkv = np.concatenate([np.asarray(bk, F32),
                          np.asarray(bv, F32)])[None, :].astype(BF16)
    iota = np.tile(np.arange(128, dtype=F32)[None, :], (128, 1)).astype(BF16)

    common = {
        "xT": xT_pad, "Wkv": Wkv, "Wq_": np.asarray(Wq, F32).astype(BF16),
        "We65": We65, "Wo_": np.asarray(Wo, F32), "bkv": bkv,
        "bq_": np.asarray(bq, F32)[None, :].astype(BF16),
        "bo_row": np.asarray(bo, F32)[None, :],
        "ones_row": np.ones((1, 512), F32),
        "ones16_row": np.ones((1, 128), BF16),
        "ident16_in": np.eye(128, dtype=F32).astype(BF16),
        "iota_in": iota, "ident_in": np.eye(128, dtype=F32),
    }
    in_maps = []
    T = plan["T"]
    for c in range(cfg.NCORES):
        xTq = np.zeros((128, cfg.Q_ROWS), BF16)
        lo = c * cfg.NPC
        xTq[:, :cfg.NPC] = x[lo:lo + cfg.NPC].T.astype(BF16)
        eids = plan["eids"][c].reshape(-1)
        ea_rows = np.zeros((T * 128, ED), BF16)
        valid = eids >= 0
        ea_rows[valid] = ea[eids[valid]].astype(BF16)
        ea_t = np.zeros((T, 65, 128), BF16)
        ea_t[:, :ED, :] = ea_rows.reshape(T, 128, ED).transpose(0, 2, 1)
        ea_t[:, ED, :] = 1.0
        in_maps.append(dict(common,
                            xTq=np.ascontiguousarray(xTq),
                            ea_all=np.ascontiguousarray(ea_t),
                            ld_in=np.ascontiguousarray(
                                plan["ld_all"][c].astype(BF16)),
                            kvidx_in=np.ascontiguousarray(plan["kvidx"][c]),
                            qidx_in=np.ascontiguousarray(plan["qidx"][c])))
    return in_maps


def _assemble(results, cfg=CFG):
    out = np.empty((cfg.N, D), F32)
    for c in range(cfg.NCORES):
        out[c * cfg.NPC:(c + 1) * cfg.NPC] = \
            np.asarray(results[c]["outT"])[:, :cfg.NPC].T
    return out


def kernel(x, edge_attr, Wq, bq, Wk, bk, Wv, bv, We, be, Wo, bo, edge_index):
    from concourse import bass_utils

    cfg = CFG
    plan = _preprocess(np.asarray(edge_index), cfg)
    nc = _build_program(plan, cfg)
    in_maps = _make_inputs(plan, x, edge_attr, Wq, bq, Wk, bk, Wv, bv,
                           We, be, Wo, bo, cfg)
    res = bass_utils.run_bass_kernel_spmd(nc, in_maps,
                                          core_ids=list(range(cfg.NCORES)))
    return _assemble(res.results, cfg)

